# revision 40
# baseline (speedup 1.0000x reference)
"""Trainium2 Bass kernel for PixelPropagationModule (per-pixel self-attention).

V4 "transposed-output" dataflow.  Math per sample b (B=8, C=256, CI=64,
N=H*W=3136):
    Q = Wq @ x + bq            [CI, N]
    K = Wk @ x + bk            [CI, N]
    V = gamma * Wv @ x         [C,  N]   (bv deferred into the residual)
    score[i, j] = sum_o Q[o, i] K[o, j]          (N x N)
    att = softmax(score, axis=j)
    out^T = att @ V^T          [N, C]  (computed TRANSPOSED, i on partitions)
    result = out^T / s_i + xf^T                  (xf = x + gamma*bv)

Sharding: pure data parallel, one sample per NeuronCore (B == 8 == n_cores).

Device dataflow (per core):
  - Q/K projections: fp8 DoubleRow matmuls (contraction C=256 as [128,2]),
    Q duplicated on both partition halves, K split even/odd chunks across
    partition halves.  Bias adds ride on DVE (tensor_scalar), keeping ACT
    free for the softmax exp stream.
  - V projection: ONE fp8 DoubleRow matmul per 128-pixel chunk
    (lhsT = x-chunk [128,2,jsz], rhs = WvT [128,2,256]) -> V^T tiles; the
    psum->SBUF copies are split between ACT and DVE to balance the phase.
  - score: 64x128 dual-PE-tile pairs (T0: partitions 0-63 / T8: 64-127),
    13 pairs cover the 25 j-chunks; exp on ACT -> bf16 att [j, i] tiles.
  - out^T: att is the STATIONARY operand.  For each (i-block of 128, j-chunk):
    matmul(lhsT=att[j, i-block], rhs=V^T[j, 0:256]) accumulates
    out^T[i-block, 0:256]; a sibling 1-column matmul with a ones vector
    (same stationary -> no extra weight load) accumulates the softmax
    denominator s_i into a per-block psum column.  This removes the entire
    DVE accumulation chain, the po_lo/po_hi split, the ones-matmul s-reduce
    and the gpsimd endgame of the baseline.
  - endgame per i-block: DVE reciprocal of the s column -> inv [i,1], then a
    single fused DVE scalar_tensor_tensor: out = psum[:,0:256]*inv + xf^T.
    Per-i values are per-PARTITION here, so no broadcast tricks needed.
  - group 0's attention pairs are interleaved INTO the projection loop
    (proj chunk t -> score pairs 2t, 2t+1), so the PE fills the DMA-paced
    projection phase with real attention work; out^T matmuls trail their
    pair via a group-tagged global queue that crosses group boundaries
    (previous-group leftovers drain first, own pops are held back until
    the previous endgame's psum drain clears the banks), and each group's
    endgame is deferred into the next group's early pair slots.
  - V^T chunks 0-11 borrow the ps_b banks (group 0's out accumulators see
    their first write only much later), so the K projection owns the pv
    bank and the exp stream is never projection-gated; Q projections for
    groups 1+ are emitted just-in-time from inside the previous group.
  - the 64-wide tail group packs 4 score pairs per psum tile / exp to
    amortize the ACT access latency (~0.6us shorter drain tail).
  - softmax runs without max subtraction (|score| <= ~40: exp safe in f32);
    normalization is exact via the f32 psum s columns.
  - PSUM start=True clears the whole 2KiB BANK (not just the written
    region), so wherever two accumulation regions share a bank only the
    first-emitted region raises start; the later regions' first matmuls
    land on cleared has_written bits and overwrite cleanly.

PSUM budget (8 banks of 2KiB/partition):
    ps_a pool: 2 x [128, 1024] f32 (score pairs; also the chunk-0 Q
               projection and PE warm-up)                    = 4 banks
    ps_b pool: 2 x [128, 512] f32 (out^T accumulators, 2 i-blocks each;
               V-projection psum for chunks 0-11 early on)   = 2 banks
    ps_s pool: 1 x [128, 4]   f32 (s columns, 1 per i-block) = 1 bank
    ps_v pool: 1 x [128, 512] f32 (V-proj psum for chunks 12+;
               just-in-time Q projections for groups 1+)     = 1 bank
"""

import numpy as np
import ml_dtypes

import bass_rust as _bass_rust

import concourse.bass as bass
import concourse.mybir as mybir
import concourse.tile as tile
from concourse.bass_utils import run_bass_kernel_spmd

BF16 = mybir.dt.bfloat16
F32 = mybir.dt.float32
FP8 = mybir.dt.float8e4
NP_BF16 = ml_dtypes.bfloat16
NP_FP8 = ml_dtypes.float8_e4m3   # TRN FP8_EXP4: max +-240, has inf
AF = mybir.ActivationFunctionType
ALU = mybir.AluOpType

B, C, H, W = 8, 256, 56, 56
CI = 64
N = H * W            # 3136
NP_ = 3200           # N padded to 25 * 128 (transposed-layout row padding)
NB = 25              # 128-row i/j blocks (24 full + 1 of 64)
NCORES = 8
PFD = 512            # projection chunk: 6 * 512 + 64 = 3136
NJ = 25              # j-chunks: 24 x 128 + 1 x 64
NPAIR = 13           # score pairs: 12 full + 1 single (chunk 24, T0 only)
OFF2 = 512           # second-half element offset inside [128, 1024] psum
FDMAX = 512
GROUPS = [(g * 512, 512) for g in range(6)] + [(3072, 64)]


def build_kernel(n_repeat: int = 1, hw_loop: bool = False,
                 sim_shrink: bool = False) -> bass.Bass:
    # sim_shrink: cost-model aid only -- TimelineSim charges concurrent
    # 64x128-tile matmuls serially, so shrink the T8-side score matmuls to
    # 16-wide (deps preserved, ~zero sim cost) to approximate the real
    # dual-tile concurrency.  The out^T matmuls use the full 128-partition
    # array and are charged in full.
    SW = 16 if sim_shrink else None
    nc = bass.Bass()

    xb_d = nc.declare_dram_parameter("xb", [C, N], FP8, isOutput=False)
    xfT_d = nc.declare_dram_parameter("xfT", [NP_, C], F32, isOutput=False)
    wqkv_d = nc.declare_dram_parameter("wqkv2T", [C, 512], FP8, isOutput=False)
    bqk_d = nc.declare_dram_parameter("bqk2", [128, 2], F32, isOutput=False)
    out_d = nc.declare_dram_parameter("out", [NP_, C], BF16, isOutput=True)

    xb_r = xb_d[:].rearrange("(o p) n -> p o n", p=128)      # [128, 2, N]
    xfT_r = xfT_d[:].rearrange("(b p) c -> p b c", p=128)    # [128, 25, C]
    out_r = out_d[:].rearrange("(b p) c -> p b c", p=128)    # [128, 25, C]

    with tile.TileContext(nc) as tc:
        with (
            tc.tile_pool(name="const", bufs=1) as cpool,
            tc.tile_pool(name="data", bufs=1) as dpool,
            tc.tile_pool(name="att", bufs=9) as apool,
            tc.tile_pool(name="outp", bufs=2) as opool,
            tc.tile_pool(name="misc", bufs=8) as mpool,
            tc.tile_pool(name="ps_a", bufs=2, space="PSUM") as ps_a,
            tc.tile_pool(name="ps_b", bufs=2, space="PSUM") as ps_b,
            tc.tile_pool(name="ps_s", bufs=1, space="PSUM") as ps_s,
            tc.tile_pool(name="ps_v", bufs=1, space="PSUM") as ps_v,
        ):
            # warm the PE HAM clock gate during the initial DMA wait: dummy
            # matmuls on a scratch tile (results never read; Pool memset is
            # the fastest path to a readable tile at kernel start)
            scratch_sb = cpool.tile([128, 256], BF16, name="scratch_sb")
            nc.gpsimd.memset(scratch_sb[:], 0.0)
            pwarm = ps_a.tile([128, 1024], F32, tag="ps_a")
            for wi in range(2):
                nc.tensor.matmul(pwarm[:, 0:256], lhsT=scratch_sb[:, 0:128],
                                 rhs=scratch_sb[:], start=True, stop=True)

            # ones vector for the s-column sibling matmuls
            ones_sb = cpool.tile([128, 1], BF16, name="ones_sb")
            nc.vector.memset(ones_sb[:], 1.0)

            # ---- weights / constants (wq|wk fused into one DMA; bq|bk too).
            # Order: wqk first, then xb chunk 0, so the first Q/K projection
            # can start as early as possible (HWDGE serializes at ~625ns/DMA).
            xb_sb = dpool.tile([128, 2, N], FP8, name="xb_sb")
            xb_edges = [0, 512, 1536, 2560, N]
            nc.sync.dma_start(xb_sb[:, :, 0:512], xb_r[:, :, 0:512])

            # wq|wk|wv fused into ONE DMA: each HWDGE slot saved moves every
            # later xb chunk 625ns earlier (HWDGE generation serializes)
            wqkv_sb = cpool.tile([128, 2, 512], FP8, name="wqkv_sb")
            nc.sync.dma_start(wqkv_sb[:], wqkv_d[:].rearrange("(o p) m -> p o m", p=128))
            bqk_sb = cpool.tile([128, 2], F32, name="bqk_sb")
            nc.sync.dma_start(bqk_sb[:], bqk_d[:])

            for e0, e1 in zip(xb_edges[1:-1], xb_edges[2:]):
                nc.sync.dma_start(xb_sb[:, :, e0:e1], xb_r[:, :, e0:e1])

            # residual input (transposed, padded): needed only from group 0's
            # endgame on, so emit after xb to not steal early DMA bandwidth
            xfT_sb = dpool.tile([128, NB, C], F32, name="xfT_sb")
            nc.sync.dma_start(xfT_sb[:], xfT_r)

            # q duplicated on both partition halves; k pairs split even/odd
            q2_sb = dpool.tile([128, N], BF16, name="q2_sb")
            k2_sb = dpool.tile([128, 13 * 128], BF16, name="k2_sb")
            # pair 12 has no odd chunk and its T0 weight slice is read 128
            # wide (cols 1600:1664 never written) -> zero once
            nc.vector.memset(k2_sb[:, 1600:1664], 0.0)
            # V^T tiles: vt_sb[p, jt, c] = gamma*V[c, jt*128+p]
            vt_sb = dpool.tile([128, NJ, C], BF16, name="vt_sb")

            def _emit_body():
                PENDING = []   # queued (gid, out^T-matmul closure), FIFO
                DEFER = []     # queued (gid, endgame closure)

                def pump(g, t):
                    # Foreign (previous-group) closures drain first, max 2 per
                    # pair; a group's endgame is emitted as soon as its last
                    # out^T matmuls have been; own closures are held back so
                    # the previous endgame's psum drain (DVE) clears the
                    # pb/sps banks before this group's start=True matmuls.
                    # Group 0 holds longer: its first out^T write must come
                    # after the last borrowed-ps_b V-projection copy.
                    npop = 0
                    while PENDING and PENDING[0][0] < g and npop < 2:
                        PENDING.pop(0)[1]()
                        npop += 1
                    if DEFER and not (PENDING and PENDING[0][0] <= DEFER[0][0]):
                        DEFER.pop(0)[1]()
                    if npop == 0:
                        hold = (max(3, 7 - max(0, t - 5)) if g == 0 else
                                3 if g == len(GROUPS) - 1 else 3)
                        while len(PENDING) > hold:
                            PENDING.pop(0)[1]()

                # ---- per-group pair emission (generator: one yield/pair) --
                def group_pairs(g, i0, fd):
                    isl = slice(i0, i0 + fd)
                    nb = (fd + 127) // 128
                    gb0 = i0 // 128
                    pbs = [ps_b.tile([128, 512], F32, tag="ps_b",
                                     name=f"pb{g}_{h}") for h in range(2)]
                    sps = ps_s.tile([128, 4], F32, tag="ps_s", name=f"sps{g}")

                    def out_mms(t, pbs=pbs, sps=sps, fd=fd, nb=nb):
                        att, pi = att_tiles.pop((g, t))
                        for par in (0, 1):
                            jt = 2 * t + par
                            if jt >= NJ:
                                continue
                            pl = 64 if jt == NJ - 1 else 128
                            for b_ in range(nb):
                                mb = min(128, fd - b_ * 128)
                                a0 = pi * fd + b_ * 128
                                lw = att[0:pl, par, a0:a0 + mb]
                                # start=True clears the whole PSUM BANK, so
                                # only the bank's first region issues it; the
                                # later regions' first writes land on cleared
                                # has_written bits and overwrite cleanly
                                # (emission order guarantees the bank-clear
                                # precedes them).
                                nc.tensor.matmul(
                                    pbs[b_ // 2][0:mb,
                                                 (b_ % 2) * C:(b_ % 2) * C + C],
                                    lhsT=lw, rhs=vt_sb[0:pl, jt, :],
                                    start=(jt == 0 and b_ % 2 == 0),
                                    stop=(jt == NJ - 1))
                                nc.tensor.matmul(
                                    sps[0:mb, b_:b_ + 1],
                                    lhsT=lw, rhs=ones_sb[0:pl],
                                    start=(jt == 0 and b_ == 0),
                                    stop=(jt == NJ - 1))

                    def endgame(pbs=pbs, sps=sps, fd=fd, nb=nb, gb0=gb0,
                                last=(g == len(GROUPS) - 1)):
                        # pb/sps banks gate the next group's start=True
                        # matmuls: the 1/s scale-mults (psum reads) run on
                        # DVE; the residual adds (SBUF-only) go to the
                        # otherwise-idle Pool engine.  The last group fuses
                        # both into one DVE op to shorten the final drain.
                        outg = opool.tile([128, 4, C], BF16, tag="outg")
                        invs = []
                        for b_ in range(nb):
                            mb = min(128, fd - b_ * 128)
                            inv = mpool.tile([128, 1], F32, tag="inv")
                            nc.vector.reciprocal(inv[0:mb],
                                                 sps[0:mb, b_:b_ + 1])
                            invs.append(inv)
                        for b_ in range(nb):
                            mb = min(128, fd - b_ * 128)
                            pbv = pbs[b_ // 2][0:mb,
                                               (b_ % 2) * C:(b_ % 2) * C + C]
                            if last:
                                nc.vector.scalar_tensor_tensor(
                                    outg[0:mb, b_, :], pbv, invs[b_][0:mb],
                                    xfT_sb[0:mb, gb0 + b_, :],
                                    op0=ALU.mult, op1=ALU.add)
                            else:
                                nc.vector.tensor_scalar_mul(
                                    outg[0:mb, b_, :], pbv, invs[b_][0:mb])
                        if not last:
                            for b_ in range(nb):
                                mb = min(128, fd - b_ * 128)
                                nc.gpsimd.tensor_add(
                                    outg[0:mb, b_, :], outg[0:mb, b_, :],
                                    xfT_sb[0:mb, gb0 + b_, :])
                        nc.sync.dma_start(out_r[:, gb0:gb0 + nb, :],
                                          outg[:, 0:nb, :])

                    # narrow groups (fd <= 128) pack NPP score pairs per
                    # psum tile / exp instruction to amortize the ACT access
                    # latency; only the first pair of each psum BANK issues
                    # start=True (bank-granular clear), later pairs overwrite
                    # onto cleared has_written bits.
                    npp = max(1, 256 // fd) if fd <= 128 else 1
                    order = list(range(NPAIR))
                    ps = att = None
                    for pos, t in enumerate(order):
                        lastp = t == NPAIR - 1
                        pi = pos % npp
                        pv12 = False
                        if pi == 0:
                            ps = ps_a.tile([128, 1024], F32, tag="ps_a")
                        o0 = pi * fd
                        nc.tensor.matmul(ps[:, o0:o0 + fd],
                                         lhsT=k2_sb[0:64, t * 128:(t + 1) * 128],
                                         rhs=q2_sb[0:64, isl],
                                         start=(pi == 0), stop=True)
                        if not lastp:
                            w8 = min(SW or fd, fd)
                            nc.tensor.matmul(ps[:, OFF2 + o0:OFF2 + o0 + w8],
                                             lhsT=k2_sb[64:128, t * 128:(t + 1) * 128],
                                             rhs=q2_sb[64:128, i0:i0 + w8],
                                             start=(pi == 0), stop=True)
                        pump(g, t)
                        flush = pi == npp - 1 or lastp
                        if flush:
                            att = apool.tile([128, 2, FDMAX], BF16, tag="att")
                            nf = (pi + 1) * fd
                            if pv12:
                                nc.scalar.activation(att[0:64, 0, 0:fd],
                                                     ps[0:64, 0:fd], AF.Exp)
                            else:
                                psv = ps[:].rearrange(
                                    "p (h x) -> p h x", h=2)[:, :, 0:nf]
                                if not lastp:
                                    nc.scalar.activation(att[:, :, 0:nf], psv,
                                                         AF.Exp)
                                else:
                                    # chunk 24: only the T0 half matters
                                    nc.scalar.activation(
                                        att[0:64, 0, o0:o0 + fd],
                                        psv[0:64, 0, o0:o0 + fd], AF.Exp)
                            for pp in range(pos - pi, pos + 1):
                                tt = order[pp]
                                att_tiles[(g, tt)] = (att, pp % npp)
                                PENDING.append((g, lambda tt=tt: out_mms(tt)))
                        if pos == 6 and g + 1 < len(GROUPS):
                            emit_qproj(g + 1)
                        yield
                    DEFER.append((g, endgame))

                att_tiles = {}

                def emit_qproj(t):
                    # Q projection for i-chunk t, just-in-time: emitted from
                    # inside group t-1 (pv pool is idle after the projection
                    # phase; DVE has ample slack in steady groups).  Keeps
                    # the score-pair psum pool free of projection traffic.
                    # Chunk 0 rides in the pre-pair ps_a pool so the K
                    # projection owns the pv bank from the very start.
                    i0, fd = GROUPS[t]
                    if t == 0:
                        pvq = ps_a.tile([128, 1024], F32, tag="ps_a",
                                        name="pvq0")
                    else:
                        pvq = ps_v.tile([128, 512], F32, tag="ps_v",
                                        name="pvq")
                    nc.tensor.matmul(pvq[:, 0:fd], lhsT=wqkv_sb[:, :, 0:128],
                                     rhs=xb_sb[:, :, i0:i0 + fd],
                                     start=True, stop=True,
                                     perf_mode=mybir.MatmulPerfMode.DoubleRow)
                    nc.vector.tensor_scalar_add(q2_sb[:, i0:i0 + fd],
                                                pvq[:, 0:fd], bqk_sb[:, 0:1])

                # ---- projections, interleaved with group 0's pairs --------
                # K and V go through the pv psum bank so the score pairs own
                # the ps_a pool outright (gap-free exp stream); Q is emitted
                # here only for chunk 0 (later chunks ride in groups 0..5).
                # pre-allocate the borrowed ps_b tiles so they take the
                # rotation slots BEFORE group 0's out-accumulators
                pvbs = [ps_b.tile([128, 512], F32, tag="ps_b", name="pvb")
                        for _ in range(6)]
                g0 = group_pairs(0, GROUPS[0][0], GROUPS[0][1])
                vt_done = 0
                nvt_copy = 0
                carry = []
                for t in range(7):
                    w = PFD if t < 6 else 64
                    sl = slice(t * PFD, t * PFD + w)
                    if t == 0:
                        emit_qproj(0)
                    pvk = ps_v.tile([128, 512], F32, tag="ps_v", name="pvk")
                    nc.tensor.matmul(pvk[:, 0:w], lhsT=wqkv_sb[:, :, 128:256],
                                     rhs=xb_sb[:, :, sl], start=True, stop=True,
                                     perf_mode=mybir.MatmulPerfMode.DoubleRow)
                    if t < 6:
                        # K chunk covers j-chunks 4t..4t+3 = pairs 2t, 2t+1.
                        # even chunks (blocks 0,2) -> partitions 0-63;
                        # odd chunks (blocks 1,3) -> partitions 64-127.
                        pk = pvk[:, 0:PFD].rearrange(
                            "p (c two x) -> p two c x", two=2, x=128)
                        ksl = slice(t * 256, t * 256 + 256)
                        kd = k2_sb[:, ksl].rearrange("p (c x) -> p c x", x=128)
                        nc.vector.tensor_scalar_add(kd[0:64], pk[0:64, 0],
                                                    bqk_sb[0:64, 1:2])
                        nc.vector.tensor_scalar_add(kd[64:128], pk[64:128, 1],
                                                    bqk_sb[64:128, 1:2])
                    else:
                        # tail: j-chunk 24 (even, pair 12, T0 only)
                        nc.vector.tensor_scalar_add(k2_sb[0:64, 1536:1600],
                                                    pvk[0:64, 0:64],
                                                    bqk_sb[0:64, 1:2])
                    # V^T tiles, two chunks per psum tile, emitted BETWEEN
                    # group 0's pair emissions.  Early chunks (0-11) borrow
                    # the ps_b banks (group 0's out-accumulators see their
                    # first write only ~7us later), so the K projection owns
                    # the pv bank outright and the score pairs -- and hence
                    # the exp stream -- are never K-gated.
                    vt_avail = min(20, (t + 1) * 4) if t < 6 else NJ

                    def emit_v(j_lo, j_hi):
                        jt = j_lo
                        while jt < j_hi:
                            npair = 2 if jt + 1 < j_hi else 1
                            if jt < 12:
                                pv = pvbs.pop(0)
                            else:
                                pv = ps_v.tile([128, 512], F32, tag="ps_v",
                                               name="pv")
                            for u in range(npair):
                                jsz = 128 if jt + u < NJ - 1 else 64
                                j0 = (jt + u) * 128
                                nc.tensor.matmul(
                                    pv[:jsz, u * C:(u + 1) * C],
                                    lhsT=xb_sb[:, :, j0:j0 + jsz],
                                    rhs=wqkv_sb[:, :, 256:512], start=True, stop=True,
                                    perf_mode=mybir.MatmulPerfMode.DoubleRow)
                            if npair == 2:
                                dst = vt_sb[:, jt:jt + 2, :]
                                src = pv[:].rearrange("p (u c) -> p u c", u=2)
                            else:
                                jsz = 128 if jt < NJ - 1 else 64
                                dst = vt_sb[:jsz, jt, :]
                                src = pv[:jsz, 0:C]
                            # copies are CARRIED one proj iteration so the
                            # next k-bias enters the DVE queue ahead of them
                            carry.append(
                                lambda d=dst, s=src: nc.vector.tensor_copy(d, s))
                            jt += npair

                    mid = min(vt_done + 2, vt_avail)
                    next(g0, None)          # pair 2t
                    for c in carry:
                        c()
                    carry = []
                    emit_v(vt_done, mid)
                    next(g0, None)          # pair 2t+1
                    emit_v(mid, vt_avail)
                    vt_done = vt_avail
                for c in carry:
                    c()
                carry = []
                for _ in g0:
                    pass

                # ---- remaining i-groups ----
                for g in range(1, len(GROUPS)):
                    i0, fd = GROUPS[g]
                    for _ in group_pairs(g, i0, fd):
                        pass

                while PENDING:
                    PENDING.pop(0)[1]()
                while DEFER:
                    DEFER.pop(0)[1]()

            if hw_loop:
                with tc.For_i(0, n_repeat):
                    _emit_body()
            else:
                for _rep in range(n_repeat):
                    _emit_body()

    # TRN2 allows at most one semaphore wait per instruction; Tile can emit
    # more. Split them (EventSemaphore chains) like Bacc.compile() does.
    _bass_rust.move_matmul_waits_to_ldweights(nc.m)
    _bass_rust.generate_event_semaphores(nc)
    return nc


_CACHED = {}


def _get_kernel(n_repeat: int = 1) -> bass.Bass:
    if n_repeat not in _CACHED:
        _CACHED[n_repeat] = build_kernel(n_repeat)
    return _CACHED[n_repeat]


def make_in_maps(x, Wq, bq, Wk, bk, Wv, bv, gamma):
    x = np.asarray(x, dtype=np.float32)
    Wq = np.asarray(Wq, dtype=np.float32)
    bq = np.asarray(bq, dtype=np.float32)
    Wk = np.asarray(Wk, dtype=np.float32)
    bk = np.asarray(bk, dtype=np.float32)
    Wv = np.asarray(Wv, dtype=np.float32)
    bv = np.asarray(bv, dtype=np.float32)
    g = float(np.asarray(gamma, dtype=np.float32).reshape(-1)[0])

    def q8(a):
        return np.clip(a, -240, 240).astype(NP_FP8)

    wqkv2T = np.ascontiguousarray(
        q8(np.concatenate([Wq.T, Wq.T, Wk.T, Wk.T, (g * Wv).T],
                          axis=1)))                            # [C, 512]
    bqk2 = np.ascontiguousarray(
        np.stack([np.concatenate([bq, bq]),
                  np.concatenate([bk, bk])], axis=1))          # [128, 2] f32

    xf = x.reshape(B, C, N) + (g * bv)[None, :, None]          # x + gamma*bv
    xfT = np.zeros((B, NP_, C), np.float32)
    xfT[:, :N, :] = xf.transpose(0, 2, 1)
    xbf = np.ascontiguousarray(q8(x.reshape(B, C, N)))

    in_maps = []
    for b in range(B):
        in_maps.append({
            "xb": xbf[b],
            "xfT": np.ascontiguousarray(xfT[b]),
            "wqkv2T": wqkv2T,
            "bqk2": bqk2,
        })
    return in_maps


def kernel(x, Wq, bq, Wk, bk, Wv, bv, gamma):
    in_maps = make_in_maps(x, Wq, bq, Wk, bk, Wv, bv, gamma)
    nc = _get_kernel(1)
    res = run_bass_kernel_spmd(nc, in_maps, core_ids=list(range(NCORES)))
    out = np.stack([res.results[b]["out"][:N].T for b in range(B)], axis=0)
    return np.ascontiguousarray(out.reshape(B, C, H, W).astype(np.float32))
